# revision 19
# baseline (speedup 1.0000x reference)
"""Cross-attention kernel for 8 TRN2 NeuronCores.

Reference computation (per problem spec):
    q = (x @ Wq)  [B=4, N=4096, D=1024] -> heads [B, 16, N, 64]
    k = (context @ Wk), v = (context @ Wv)   context [B, M=256, 768]
    out = softmax(q k^T / 8 + mask) v   -> [B, N, D] @ Wo

Sharding: the 16384 query rows (B*N) are split evenly across the 8 cores
(2048 rows each, each shard living inside one batch). K/V are computed
redundantly per core from that core's batch context (only ~0.8 GFLOP) so no
collectives are needed; each core produces its own 2048 output rows and the
host concatenates them.

On-core dataflow (bf16 matmuls, fp32 PSUM), pipelined over 512-row chunks:
  - host tensors arrive pre-shuffled partition-major ([128, ...]) so every
    big DMA is 128 large contiguous descriptors (descriptor throughput,
    not bytes, limits the rings here)
  - x^T: PE transposes for chunks 0-1 (fills the HAM warmup window),
    DMA-transposes on the two HWDGE rings for chunks 2-3 (row-major copy
    of x is passed separately for those)
  - Q^T = Wq^T x^T; scores^T = K Q^T (kv on partitions, two heads
    row-packed in the PE); exp on ScalarE with the mask as bias
  - AV+rowsum fused: lhsT = [ones | V_h] so one matmul per (head,
    kv-block) emits the softmax denominator on partitions 0:63
    (reciprocal_approx_fast needs base partition 0) and AV on 64:127;
    the normalize multiply mixes base partitions (PSUM+SB: allowed)
  - out rows = O^T.T Wo, evicted bf16 and stored once per chunk
"""

import sys

for _p in ("/opt/trn_rl_repo",):
    if _p not in sys.path:
        sys.path.insert(0, _p)

import numpy as np

import concourse.bass as bass
import concourse.mybir as mybir
import concourse.tile as tile
from concourse.masks import make_identity
from concourse import bacc
from concourse.bass_utils import run_bass_kernel_spmd

ts = bass.ts

N_CORES = 8
B, N, D = 4, 4096, 1024
CTX = 768
M = 256          # kv length
H, HD = 16, 64   # heads, head dim
NQ = (B * N) // N_CORES   # 2048 query rows per core
QCH = 512                 # q chunk (free dim of most matmuls)
NQC = NQ // QCH           # 4 q chunks
NRB = NQ // 128           # 16 row-blocks
DT = D // 128             # 8 d-blocks (= head pairs)
KCH = CTX // 128          # 6 contraction chunks for context projections
F32 = mybir.dt.float32
BF16 = mybir.dt.bfloat16

SCALE = HD ** -0.5


def build_nc():
    nc = bacc.Bacc()

    x_pm = nc.declare_dram_parameter("xpm", [128, 8, D], BF16, isOutput=False)
    x_rm = nc.declare_dram_parameter("xrm", [NQ, D], BF16, isOutput=False)
    ctx_ext = nc.declare_dram_parameter("ctx", [128, 2, CTX], BF16, isOutput=False)
    maskb_ext = nc.declare_dram_parameter("maskb", [128, 2], F32, isOutput=False)
    wq_ext = nc.declare_dram_parameter("wq", [128, DT, D], BF16, isOutput=False)
    wk_ext = nc.declare_dram_parameter("wk", [128, KCH, D], BF16, isOutput=False)
    wv_ext = nc.declare_dram_parameter("wv", [128, KCH, D], BF16, isOutput=False)
    wo_ext = nc.declare_dram_parameter("wo", [128, DT, D], BF16, isOutput=False)
    out_ext = nc.declare_dram_parameter("out", [128, NRB, D], BF16, isOutput=True)

    with tile.TileContext(nc) as tc:
        # ---- persistent tensors -------------------------------------------
        identb, free_identb = tc.tile([128, 128], BF16, name="identb")
        make_identity(nc, identb)
        mask_sb, free_mask = tc.tile([128, 2], F32, name="mask_sb")
        nc.scalar.dma_start(out=mask_sb, in_=maskb_ext[:, :])

        kT, free_kT = tc.tile([128, DT, M], BF16, name="kT")
        # per (kv-block j, head h): [ones | V_h]; the ones columns make each
        # AV matmul also emit the softmax denominator on partitions 0:63
        vvx, free_vvx = tc.tile([128, 2, H, 128], BF16, name="vvx")
        x01, free_x01 = tc.tile([128, 8, D], BF16, name="x01")
        xT, free_xT = tc.tile([128, DT, NQ], BF16, name="xT")
        qT, free_qT = tc.tile([128, DT, 2, QCH], BF16, name="qT")
        oT, free_oT = tc.tile([128, DT, 2, QCH], BF16, name="oT")

        nc.vector.memset(vvx, 1.0)

        with tc.tile_pool(name="weights", bufs=1) as wpool, \
             tc.tile_pool(name="bpool", bufs=1) as bpool, \
             tc.tile_pool(name="attnp", bufs=3) as attnp, \
             tc.tile_pool(name="recp", bufs=3) as recp, \
             tc.tile_pool(name="outp", bufs=2) as outp, \
             tc.tile_pool(name="mpsum", bufs=4, space="PSUM") as mpsum:
            # ---- input DMAs: sync ring x/ctx, scalar ring weights ---------
            nc.sync.dma_start(out=x01[:, 0:4, :], in_=x_pm[:, 0:4, :])
            nc.sync.dma_start(out=x01[:, 4:8, :], in_=x_pm[:, 4:8, :])
            ctx_sb = bpool.tile([128, 2, CTX], BF16, name="ctx_sb")
            nc.sync.dma_start(out=ctx_sb, in_=ctx_ext[:, :, :])
            for k in range(DT):
                nc.sync.dma_start(
                    out=xT[:, k, ts(2, QCH)],
                    in_=x_rm[ts(2, QCH), ts(k, 128)], transpose=True,
                )
            wq_sb = wpool.tile([128, DT, D], BF16, name="wq_sb")
            nc.scalar.dma_start(out=wq_sb, in_=wq_ext[:, :, :])
            wk_sb = bpool.tile([128, KCH, D], BF16, name="wk_sb")
            nc.scalar.dma_start(out=wk_sb, in_=wk_ext[:, :, :])
            wv_sb = bpool.tile([128, KCH, D], BF16, name="wv_sb")
            nc.scalar.dma_start(out=wv_sb, in_=wv_ext[:, :, :])
            wo_sb = wpool.tile([128, DT, D], BF16, name="wo_sb")
            nc.scalar.dma_start(out=wo_sb, in_=wo_ext[:, :, :])
            for k in range(DT):
                nc.scalar.dma_start(
                    out=xT[:, k, ts(3, QCH)],
                    in_=x_rm[ts(3, QCH), ts(k, 128)], transpose=True,
                )

            # ---- x^T for chunks 0-1 (PE transposes) + Q chunk 0 first:
            # their data arrives earliest, so they fill the PE queue while
            # the K/V weights are still streaming in
            def do_xT(c):
                for rr in range(4):
                    rb = 4 * c + rr
                    for kg in range(2):
                        tp = mpsum.tile([128, 4, 128], BF16, name="tp", tag="ps")
                        for kk in range(4):
                            nc.tensor.transpose(
                                tp[:, kk, :], x01[:, rb, ts(4 * kg + kk, 128)],
                                identb,
                            )
                        nc.vector.tensor_copy(
                            xT[:, 4 * kg : 4 * kg + 4, ts(rb, 128)], tp
                        )

            def do_qproj(c):
                cc = c % 2
                for m in range(DT):
                    ps = mpsum.tile([128, QCH], F32, name="ps_q", tag="ps")
                    for k in range(DT):
                        nc.tensor.matmul(
                            ps[:, :], wq_sb[:, k, ts(m, 128)],
                            xT[:, k, ts(c, QCH)],
                            start=(k == 0), stop=(k == DT - 1),
                        )
                    nc.scalar.activation(
                        qT[:, m, cc, :], ps,
                        mybir.ActivationFunctionType.Copy,
                    )

            do_xT(0)
            do_xT(1)
            do_qproj(0)

            # ---- K/V setup ------------------------------------------------
            ctxT = bpool.tile([128, KCH, M], BF16, name="ctxT")
            for a in range(2):
                tp = mpsum.tile([128, KCH, 128], BF16, name="tp_b", tag="ps")
                for k in range(KCH):
                    nc.tensor.transpose(
                        tp[:, k, :], ctx_sb[:, a, ts(k, 128)], identb
                    )
                nc.vector.tensor_copy(ctxT[:, :, ts(a, 128)], tp)
            for m in range(DT):
                ps = mpsum.tile([128, M], F32, name="ps_k", tag="ps")
                for k in range(KCH):
                    nc.tensor.matmul(
                        ps[:, :], wk_sb[:, k, ts(m, 128)], ctxT[:, k, :],
                        start=(k == 0), stop=(k == KCH - 1),
                    )
                nc.vector.tensor_copy(kT[:, m, :], ps)
            for j in range(2):
                for n in range(2):
                    psv = mpsum.tile([128, 8, HD], F32, name="ps_v", tag="ps")
                    for k in range(KCH):
                        nc.tensor.matmul(
                            psv[:, :, :], ctxT[:, k, ts(j, 128)],
                            wv_sb[:, k, ts(n, 512)],
                            start=(k == 0), stop=(k == KCH - 1),
                        )
                    nc.vector.tensor_copy(
                        vvx[:, j, 8 * n : 8 * n + 8, HD:128], psv
                    )

            # ---- main pipeline over q chunks ------------------------------
            for c in range(NQC):
                cc = c % 2
                # attention chunk c, per head pair
                for i in range(DT):
                    av_e = mpsum.tile([128, QCH], F32, name="av_e", tag="av_e", bufs=2)
                    av_o = mpsum.tile([128, QCH], F32, name="av_o", tag="av_o", bufs=2)
                    for j in range(2):  # kv chunk
                        sc_e = mpsum.tile([128, QCH], F32, name="sc_e", tag="ps")
                        sc_o = mpsum.tile([128, QCH], F32, name="sc_o", tag="ps")
                        nc.tensor.matmul(
                            sc_e[:, :], kT[0:64, i, ts(j, 128)],
                            qT[0:64, i, cc, :],
                            start=True, stop=True, tile_position=(0, 0),
                        )
                        nc.tensor.matmul(
                            sc_o[:, :], kT[64:128, i, ts(j, 128)],
                            qT[64:128, i, cc, :],
                            start=True, stop=True, tile_position=(64, 0),
                        )
                        at_e = attnp.tile([128, QCH], BF16, name="at_e", tag="at_e")
                        at_o = attnp.tile([128, QCH], BF16, name="at_o", tag="at_o")
                        nc.scalar.activation(
                            at_e, sc_e, mybir.ActivationFunctionType.Exp,
                            bias=mask_sb[:, j : j + 1], scale=SCALE,
                        )
                        nc.scalar.activation(
                            at_o, sc_o, mybir.ActivationFunctionType.Exp,
                            bias=mask_sb[:, j : j + 1], scale=SCALE,
                        )
                        nc.tensor.matmul(
                            av_e[:, :], vvx[:, j, 2 * i, :], at_e,
                            start=(j == 0), stop=(j == 1),
                        )
                        nc.tensor.matmul(
                            av_o[:, :], vvx[:, j, 2 * i + 1, :], at_o,
                            start=(j == 0), stop=(j == 1),
                        )
                    rec_e = recp.tile([64, QCH], F32, name="rec_e", tag="rec_e")
                    rec_o = recp.tile([64, QCH], F32, name="rec_o", tag="rec_o")
                    nc.vector.reciprocal_approx_fast(rec_e, av_e[0:64, :])
                    nc.vector.reciprocal_approx_fast(rec_o, av_o[0:64, :])
                    nc.vector.tensor_mul(
                        oT[0:64, i, cc, :], av_e[64:128, :], rec_e
                    )
                    nc.vector.tensor_mul(
                        oT[64:128, i, cc, :], av_o[64:128, :], rec_o
                    )
                # next chunk's Q projection between attention and out-proj
                if c + 1 < NQC:
                    do_qproj(c + 1)
                # out rows chunk c = O^T.T @ Wo, merged bf16 store
                ob = outp.tile([128, 4, D], BF16, name="ob", tag="ob")
                for mr in range(4):
                    for n in range(2):
                        ps = mpsum.tile([128, 512], F32, name="ps_o", tag="ps")
                        for k in range(DT):
                            nc.tensor.matmul(
                                ps[:, :], oT[:, k, cc, ts(mr, 128)],
                                wo_sb[:, k, ts(n, 512)],
                                start=(k == 0), stop=(k == DT - 1),
                            )
                        nc.vector.tensor_copy(ob[:, mr, ts(n, 512)], ps)
                nc.sync.dma_start(
                    out=out_ext[:, 4 * c : 4 * c + 4, :], in_=ob
                )

        # release singles in reverse allocation order
        free_oT()
        free_qT()
        free_xT()
        free_x01()
        free_vvx()
        free_kT()
        free_mask()
        free_identb()

    nc.finalize()
    return nc


_NC_CACHE = None


def _get_nc():
    global _NC_CACHE
    if _NC_CACHE is None:
        _NC_CACHE = build_nc()
    return _NC_CACHE


def _shuffle_pm(a, nblk):
    """[nblk*128, d] -> [128, nblk, d] partition-major."""
    d = a.shape[1]
    return np.ascontiguousarray(a.reshape(nblk, 128, d).transpose(1, 0, 2))


def make_in_maps(x, context, context_mask, Wq, Wk, Wv, Wo):
    import ml_dtypes

    bf = ml_dtypes.bfloat16
    x = np.asarray(x).astype(bf)
    context = np.asarray(context).astype(bf)
    Wq = np.asarray(Wq).astype(bf)
    Wk = np.asarray(Wk).astype(bf)
    Wv = np.asarray(Wv).astype(bf)
    Wo = np.asarray(Wo).astype(bf)
    mask = np.asarray(context_mask)

    # additive exp-bias per kv position: 0 where visible, -1e9 where masked
    bias = (mask.astype(np.float32) - 1.0) * 1e9          # [B, M]
    x_flat = x.reshape(B * N, D)

    wq_s = _shuffle_pm(Wq, DT)
    wk_s = _shuffle_pm(Wk, KCH)
    wv_s = _shuffle_pm(Wv, KCH)
    wo_s = _shuffle_pm(Wo, DT)

    in_maps = []
    for c in range(N_CORES):
        b = (c * NQ) // N
        shard = x_flat[c * NQ : (c + 1) * NQ]
        in_maps.append({
            "xpm": _shuffle_pm(shard[: 8 * 128], 8),
            "xrm": np.ascontiguousarray(shard),
            "ctx": _shuffle_pm(context[b], 2),
            "maskb": np.ascontiguousarray(bias[b].reshape(2, 128).T),
            "wq": wq_s, "wk": wk_s, "wv": wv_s, "wo": wo_s,
        })
    return in_maps


def kernel(x, context, context_mask, Wq, Wk, Wv, Wo):
    nc = _get_nc()
    in_maps = make_in_maps(x, context, context_mask, Wq, Wk, Wv, Wo)
    res = run_bass_kernel_spmd(nc, in_maps, core_ids=list(range(N_CORES)))
    # out arrives partition-major bf16: [128, NRB, D] per core
    outs = []
    for c in range(N_CORES):
        o = np.asarray(res.results[c]["out"], dtype=np.float32)
        outs.append(o.transpose(1, 0, 2).reshape(NQ, D))
    return np.concatenate(outs, axis=0).reshape(B, N, D)


# revision 20
# speedup vs baseline: 1.0064x; 1.0064x over previous
"""Cross-attention kernel for 8 TRN2 NeuronCores.

Reference computation (per problem spec):
    q = (x @ Wq)  [B=4, N=4096, D=1024] -> heads [B, 16, N, 64]
    k = (context @ Wk), v = (context @ Wv)   context [B, M=256, 768]
    out = softmax(q k^T / 8 + mask) v   -> [B, N, D] @ Wo

Sharding: the 16384 query rows (B*N) are split evenly across the 8 cores
(2048 rows each, each shard living inside one batch). K/V are computed
redundantly per core from that core's batch context (only ~0.8 GFLOP) so no
collectives are needed; each core produces its own 2048 output rows and the
host concatenates them.

On-core dataflow (bf16 matmuls, fp32 PSUM), pipelined over 512-row chunks:
  - host tensors arrive pre-shuffled partition-major ([128, ...]) so every
    big DMA is 128 large contiguous descriptors (descriptor throughput,
    not bytes, limits the rings here)
  - x^T: PE transposes for chunks 0-1 (fills the HAM warmup window),
    DMA-transposes on the two HWDGE rings for chunks 2-3 (row-major copy
    of x is passed separately for those)
  - Q^T = Wq^T x^T; scores^T = K Q^T (kv on partitions, two heads
    row-packed in the PE); exp on ScalarE with the mask as bias
  - AV+rowsum fused: lhsT = [ones | V_h] so one matmul per (head,
    kv-block) emits the softmax denominator on partitions 0:63
    (reciprocal_approx_fast needs base partition 0) and AV on 64:127;
    the normalize multiply mixes base partitions (PSUM+SB: allowed)
  - out rows = O^T.T Wo, evicted bf16 and stored once per chunk
"""

import sys

for _p in ("/opt/trn_rl_repo",):
    if _p not in sys.path:
        sys.path.insert(0, _p)

import numpy as np

import concourse.bass as bass
import concourse.mybir as mybir
import concourse.tile as tile
from concourse.masks import make_identity
from concourse import bacc
from concourse.bass_utils import run_bass_kernel_spmd

ts = bass.ts

N_CORES = 8
B, N, D = 4, 4096, 1024
CTX = 768
M = 256          # kv length
H, HD = 16, 64   # heads, head dim
NQ = (B * N) // N_CORES   # 2048 query rows per core
QCH = 512                 # q chunk (free dim of most matmuls)
NQC = NQ // QCH           # 4 q chunks
NRB = NQ // 128           # 16 row-blocks
DT = D // 128             # 8 d-blocks (= head pairs)
KCH = CTX // 128          # 6 contraction chunks for context projections
F32 = mybir.dt.float32
BF16 = mybir.dt.bfloat16

SCALE = HD ** -0.5


def build_nc():
    nc = bacc.Bacc()

    x_pm = nc.declare_dram_parameter("xpm", [128, 8, D], BF16, isOutput=False)
    x_rm = nc.declare_dram_parameter("xrm", [NQ, D], BF16, isOutput=False)
    ctx_ext = nc.declare_dram_parameter("ctx", [128, 2, CTX], BF16, isOutput=False)
    maskb_ext = nc.declare_dram_parameter("maskb", [128, 2], F32, isOutput=False)
    wq_ext = nc.declare_dram_parameter("wq", [128, DT, D], BF16, isOutput=False)
    wk_ext = nc.declare_dram_parameter("wk", [128, KCH, D], BF16, isOutput=False)
    wv_ext = nc.declare_dram_parameter("wv", [128, KCH, D], BF16, isOutput=False)
    wo_ext = nc.declare_dram_parameter("wo", [128, DT, D], BF16, isOutput=False)
    out_ext = nc.declare_dram_parameter("out", [128, NRB, D], BF16, isOutput=True)

    with tile.TileContext(nc) as tc:
        # ---- persistent tensors -------------------------------------------
        identb, free_identb = tc.tile([128, 128], BF16, name="identb")
        make_identity(nc, identb)
        mask_sb, free_mask = tc.tile([128, 2], F32, name="mask_sb")
        nc.scalar.dma_start(out=mask_sb, in_=maskb_ext[:, :])

        kT, free_kT = tc.tile([128, DT, M], BF16, name="kT")
        # per (kv-block j, head h): [ones | V_h]; the ones columns make each
        # AV matmul also emit the softmax denominator on partitions 0:63
        vvx, free_vvx = tc.tile([128, 2, H, 128], BF16, name="vvx")
        x01, free_x01 = tc.tile([128, 8, D], BF16, name="x01")
        xT, free_xT = tc.tile([128, DT, NQ], BF16, name="xT")
        qT, free_qT = tc.tile([128, DT, 2, QCH], BF16, name="qT")
        oT, free_oT = tc.tile([128, DT, 2, QCH], BF16, name="oT")

        nc.vector.memset(vvx, 1.0)

        with tc.tile_pool(name="weights", bufs=1) as wpool, \
             tc.tile_pool(name="bpool", bufs=1) as bpool, \
             tc.tile_pool(name="attnp", bufs=3) as attnp, \
             tc.tile_pool(name="recp", bufs=3) as recp, \
             tc.tile_pool(name="outp", bufs=2) as outp, \
             tc.tile_pool(name="mpsum", bufs=4, space="PSUM") as mpsum:
            # ---- input DMAs: sync ring x/ctx, scalar ring weights.
            # high_priority pins them ahead of the (bulky, low-urgency)
            # transposed-x descriptor streams on the shared DMA queues.
            ctx_sb = bpool.tile([128, 2, CTX], BF16, name="ctx_sb")
            wq_sb = wpool.tile([128, DT, D], BF16, name="wq_sb")
            wk_sb = bpool.tile([128, KCH, D], BF16, name="wk_sb")
            wv_sb = bpool.tile([128, KCH, D], BF16, name="wv_sb")
            wo_sb = wpool.tile([128, DT, D], BF16, name="wo_sb")
            with tc.high_priority():
                nc.sync.dma_start(out=x01[:, 0:4, :], in_=x_pm[:, 0:4, :])
                nc.sync.dma_start(out=x01[:, 4:8, :], in_=x_pm[:, 4:8, :])
                nc.sync.dma_start(out=ctx_sb, in_=ctx_ext[:, :, :])
                nc.scalar.dma_start(out=wq_sb, in_=wq_ext[:, :, :])
                nc.scalar.dma_start(out=wk_sb, in_=wk_ext[:, :, :])
                nc.scalar.dma_start(out=wv_sb, in_=wv_ext[:, :, :])
                nc.scalar.dma_start(out=wo_sb, in_=wo_ext[:, :, :])
            for k in range(DT):
                nc.sync.dma_start(
                    out=xT[:, k, ts(2, QCH)],
                    in_=x_rm[ts(2, QCH), ts(k, 128)], transpose=True,
                )
            for k in range(DT):
                nc.scalar.dma_start(
                    out=xT[:, k, ts(3, QCH)],
                    in_=x_rm[ts(3, QCH), ts(k, 128)], transpose=True,
                )

            # ---- x^T for chunks 0-1 (PE transposes) + Q chunk 0 first:
            # their data arrives earliest, so they fill the PE queue while
            # the K/V weights are still streaming in
            def do_xT(c):
                for rr in range(4):
                    rb = 4 * c + rr
                    for kg in range(2):
                        tp = mpsum.tile([128, 4, 128], BF16, name="tp", tag="ps")
                        for kk in range(4):
                            nc.tensor.transpose(
                                tp[:, kk, :], x01[:, rb, ts(4 * kg + kk, 128)],
                                identb,
                            )
                        nc.vector.tensor_copy(
                            xT[:, 4 * kg : 4 * kg + 4, ts(rb, 128)], tp
                        )

            def do_qproj(c):
                cc = c % 2
                for m in range(DT):
                    ps = mpsum.tile([128, QCH], F32, name="ps_q", tag="ps")
                    for k in range(DT):
                        nc.tensor.matmul(
                            ps[:, :], wq_sb[:, k, ts(m, 128)],
                            xT[:, k, ts(c, QCH)],
                            start=(k == 0), stop=(k == DT - 1),
                        )
                    nc.scalar.activation(
                        qT[:, m, cc, :], ps,
                        mybir.ActivationFunctionType.Copy,
                    )

            do_xT(0)
            do_xT(1)
            do_qproj(0)

            # ---- K/V setup ------------------------------------------------
            ctxT = bpool.tile([128, KCH, M], BF16, name="ctxT")
            for a in range(2):
                tp = mpsum.tile([128, KCH, 128], BF16, name="tp_b", tag="ps")
                for k in range(KCH):
                    nc.tensor.transpose(
                        tp[:, k, :], ctx_sb[:, a, ts(k, 128)], identb
                    )
                nc.vector.tensor_copy(ctxT[:, :, ts(a, 128)], tp)
            for m in range(DT):
                ps = mpsum.tile([128, M], F32, name="ps_k", tag="ps")
                for k in range(KCH):
                    nc.tensor.matmul(
                        ps[:, :], wk_sb[:, k, ts(m, 128)], ctxT[:, k, :],
                        start=(k == 0), stop=(k == KCH - 1),
                    )
                nc.vector.tensor_copy(kT[:, m, :], ps)
            for j in range(2):
                for n in range(2):
                    psv = mpsum.tile([128, 8, HD], F32, name="ps_v", tag="ps")
                    for k in range(KCH):
                        nc.tensor.matmul(
                            psv[:, :, :], ctxT[:, k, ts(j, 128)],
                            wv_sb[:, k, ts(n, 512)],
                            start=(k == 0), stop=(k == KCH - 1),
                        )
                    nc.vector.tensor_copy(
                        vvx[:, j, 8 * n : 8 * n + 8, HD:128], psv
                    )

            # ---- main pipeline over q chunks ------------------------------
            for c in range(NQC):
                cc = c % 2
                # attention chunk c, per head pair
                for i in range(DT):
                    av_e = mpsum.tile([128, QCH], F32, name="av_e", tag="av_e", bufs=2)
                    av_o = mpsum.tile([128, QCH], F32, name="av_o", tag="av_o", bufs=2)
                    for j in range(2):  # kv chunk
                        sc_e = mpsum.tile([128, QCH], F32, name="sc_e", tag="ps")
                        sc_o = mpsum.tile([128, QCH], F32, name="sc_o", tag="ps")
                        nc.tensor.matmul(
                            sc_e[:, :], kT[0:64, i, ts(j, 128)],
                            qT[0:64, i, cc, :],
                            start=True, stop=True, tile_position=(0, 0),
                        )
                        nc.tensor.matmul(
                            sc_o[:, :], kT[64:128, i, ts(j, 128)],
                            qT[64:128, i, cc, :],
                            start=True, stop=True, tile_position=(64, 0),
                        )
                        at_e = attnp.tile([128, QCH], BF16, name="at_e", tag="at_e")
                        at_o = attnp.tile([128, QCH], BF16, name="at_o", tag="at_o")
                        nc.scalar.activation(
                            at_e, sc_e, mybir.ActivationFunctionType.Exp,
                            bias=mask_sb[:, j : j + 1], scale=SCALE,
                        )
                        nc.scalar.activation(
                            at_o, sc_o, mybir.ActivationFunctionType.Exp,
                            bias=mask_sb[:, j : j + 1], scale=SCALE,
                        )
                        nc.tensor.matmul(
                            av_e[:, :], vvx[:, j, 2 * i, :], at_e,
                            start=(j == 0), stop=(j == 1),
                        )
                        nc.tensor.matmul(
                            av_o[:, :], vvx[:, j, 2 * i + 1, :], at_o,
                            start=(j == 0), stop=(j == 1),
                        )
                    rec_e = recp.tile([64, QCH], F32, name="rec_e", tag="rec_e")
                    rec_o = recp.tile([64, QCH], F32, name="rec_o", tag="rec_o")
                    nc.vector.reciprocal_approx_fast(rec_e, av_e[0:64, :])
                    nc.vector.reciprocal_approx_fast(rec_o, av_o[0:64, :])
                    nc.vector.tensor_mul(
                        oT[0:64, i, cc, :], av_e[64:128, :], rec_e
                    )
                    nc.vector.tensor_mul(
                        oT[64:128, i, cc, :], av_o[64:128, :], rec_o
                    )
                # next chunk's Q projection between attention and out-proj
                if c + 1 < NQC:
                    do_qproj(c + 1)
                # out rows chunk c = O^T.T @ Wo, merged bf16 store
                ob = outp.tile([128, 4, D], BF16, name="ob", tag="ob")
                for mr in range(4):
                    for n in range(2):
                        ps = mpsum.tile([128, 512], F32, name="ps_o", tag="ps")
                        for k in range(DT):
                            nc.tensor.matmul(
                                ps[:, :], oT[:, k, cc, ts(mr, 128)],
                                wo_sb[:, k, ts(n, 512)],
                                start=(k == 0), stop=(k == DT - 1),
                            )
                        nc.vector.tensor_copy(ob[:, mr, ts(n, 512)], ps)
                nc.sync.dma_start(
                    out=out_ext[:, 4 * c : 4 * c + 4, :], in_=ob
                )

        # release singles in reverse allocation order
        free_oT()
        free_qT()
        free_xT()
        free_x01()
        free_vvx()
        free_kT()
        free_mask()
        free_identb()

    nc.finalize()
    return nc


_NC_CACHE = None


def _get_nc():
    global _NC_CACHE
    if _NC_CACHE is None:
        _NC_CACHE = build_nc()
    return _NC_CACHE


def _shuffle_pm(a, nblk):
    """[nblk*128, d] -> [128, nblk, d] partition-major."""
    d = a.shape[1]
    return np.ascontiguousarray(a.reshape(nblk, 128, d).transpose(1, 0, 2))


def make_in_maps(x, context, context_mask, Wq, Wk, Wv, Wo):
    import ml_dtypes

    bf = ml_dtypes.bfloat16
    x = np.asarray(x).astype(bf)
    context = np.asarray(context).astype(bf)
    Wq = np.asarray(Wq).astype(bf)
    Wk = np.asarray(Wk).astype(bf)
    Wv = np.asarray(Wv).astype(bf)
    Wo = np.asarray(Wo).astype(bf)
    mask = np.asarray(context_mask)

    # additive exp-bias per kv position: 0 where visible, -1e9 where masked
    bias = (mask.astype(np.float32) - 1.0) * 1e9          # [B, M]
    x_flat = x.reshape(B * N, D)

    wq_s = _shuffle_pm(Wq, DT)
    wk_s = _shuffle_pm(Wk, KCH)
    wv_s = _shuffle_pm(Wv, KCH)
    wo_s = _shuffle_pm(Wo, DT)

    in_maps = []
    for c in range(N_CORES):
        b = (c * NQ) // N
        shard = x_flat[c * NQ : (c + 1) * NQ]
        in_maps.append({
            "xpm": _shuffle_pm(shard[: 8 * 128], 8),
            "xrm": np.ascontiguousarray(shard),
            "ctx": _shuffle_pm(context[b], 2),
            "maskb": np.ascontiguousarray(bias[b].reshape(2, 128).T),
            "wq": wq_s, "wk": wk_s, "wv": wv_s, "wo": wo_s,
        })
    return in_maps


def kernel(x, context, context_mask, Wq, Wk, Wv, Wo):
    nc = _get_nc()
    in_maps = make_in_maps(x, context, context_mask, Wq, Wk, Wv, Wo)
    res = run_bass_kernel_spmd(nc, in_maps, core_ids=list(range(N_CORES)))
    # out arrives partition-major bf16: [128, NRB, D] per core
    outs = []
    for c in range(N_CORES):
        o = np.asarray(res.results[c]["out"], dtype=np.float32)
        outs.append(o.transpose(1, 0, 2).reshape(NQ, D))
    return np.concatenate(outs, axis=0).reshape(B, N, D)


# revision 21
# speedup vs baseline: 1.0460x; 1.0394x over previous
"""Cross-attention kernel for 8 TRN2 NeuronCores.

Reference computation (per problem spec):
    q = (x @ Wq)  [B=4, N=4096, D=1024] -> heads [B, 16, N, 64]
    k = (context @ Wk), v = (context @ Wv)   context [B, M=256, 768]
    out = softmax(q k^T / 8 + mask) v   -> [B, N, D] @ Wo

Sharding: the 16384 query rows (B*N) are split evenly across the 8 cores
(2048 rows each, each shard living inside one batch). K/V are computed
redundantly per core from that core's batch context (only ~0.8 GFLOP) so no
collectives are needed; each core produces its own 2048 output rows and the
host concatenates them.

On-core dataflow (bf16 matmuls, fp32 PSUM), pipelined over 512-row chunks:
  - host tensors arrive pre-shuffled partition-major ([128, ...]) so every
    big DMA is 128 large contiguous descriptors (descriptor throughput,
    not bytes, limits the rings here)
  - x^T: PE transposes for chunks 0-1 (fills the HAM warmup window),
    DMA-transposes on the two HWDGE rings for chunks 2-3 (row-major copy
    of x is passed separately for those)
  - Q^T = Wq^T x^T; scores^T = K Q^T (kv on partitions, two heads
    row-packed in the PE); exp on ScalarE with the mask as bias
  - AV+rowsum fused: lhsT = [ones | V_h] so one matmul per (head,
    kv-block) emits the softmax denominator on partitions 0:63
    (reciprocal_approx_fast needs base partition 0) and AV on 64:127;
    the normalize multiply mixes base partitions (PSUM+SB: allowed)
  - out rows = O^T.T Wo, evicted bf16 and stored once per chunk
"""

import sys

for _p in ("/opt/trn_rl_repo",):
    if _p not in sys.path:
        sys.path.insert(0, _p)

import numpy as np

import concourse.bass as bass
import concourse.mybir as mybir
import concourse.tile as tile
from concourse.masks import make_identity
from concourse import bacc
from concourse.bass_utils import run_bass_kernel_spmd

ts = bass.ts

N_CORES = 8
B, N, D = 4, 4096, 1024
CTX = 768
M = 256          # kv length
H, HD = 16, 64   # heads, head dim
NQ = (B * N) // N_CORES   # 2048 query rows per core
QCH = 512                 # q chunk (free dim of most matmuls)
NQC = NQ // QCH           # 4 q chunks
NRB = NQ // 128           # 16 row-blocks
DT = D // 128             # 8 d-blocks (= head pairs)
KCH = CTX // 128          # 6 contraction chunks for context projections
F32 = mybir.dt.float32
BF16 = mybir.dt.bfloat16

SCALE = HD ** -0.5


def build_nc():
    nc = bacc.Bacc()

    x_pm = nc.declare_dram_parameter("xpm", [128, 8, D], BF16, isOutput=False)
    x_rm = nc.declare_dram_parameter("xrm", [NQ, D], BF16, isOutput=False)
    ctx_ext = nc.declare_dram_parameter("ctx", [128, 2, CTX], BF16, isOutput=False)
    maskb_ext = nc.declare_dram_parameter("maskb", [128, 2], F32, isOutput=False)
    wq_ext = nc.declare_dram_parameter("wq", [128, DT, D], BF16, isOutput=False)
    wk_ext = nc.declare_dram_parameter("wk", [128, KCH, D], BF16, isOutput=False)
    wv_ext = nc.declare_dram_parameter("wv", [128, KCH, D], BF16, isOutput=False)
    wo_ext = nc.declare_dram_parameter("wo", [128, DT, D], BF16, isOutput=False)
    out_ext = nc.declare_dram_parameter("out", [128, NRB, D], BF16, isOutput=True)

    with tile.TileContext(nc) as tc:
        # ---- persistent tensors -------------------------------------------
        identb, free_identb = tc.tile([128, 128], BF16, name="identb")
        make_identity(nc, identb)
        mask_sb, free_mask = tc.tile([128, 2], F32, name="mask_sb")
        nc.scalar.dma_start(out=mask_sb, in_=maskb_ext[:, :])

        kT, free_kT = tc.tile([128, DT, M], BF16, name="kT")
        # per (kv-block j, head h): [ones | V_h]; the ones columns make each
        # AV matmul also emit the softmax denominator on partitions 0:63
        vvx, free_vvx = tc.tile([128, 2, H, 128], BF16, name="vvx")
        x01, free_x01 = tc.tile([128, 8, D], BF16, name="x01")
        xT, free_xT = tc.tile([128, DT, NQ], BF16, name="xT")
        qT, free_qT = tc.tile([128, DT, 2, QCH], BF16, name="qT")
        oT, free_oT = tc.tile([128, DT, 2, QCH], BF16, name="oT")

        nc.vector.memset(vvx, 1.0)

        with tc.tile_pool(name="weights", bufs=1) as wpool, \
             tc.tile_pool(name="bpool", bufs=1) as bpool, \
             tc.tile_pool(name="attnp", bufs=3) as attnp, \
             tc.tile_pool(name="recp", bufs=3) as recp, \
             tc.tile_pool(name="outp", bufs=2) as outp, \
             tc.tile_pool(name="mpsum", bufs=4, space="PSUM") as mpsum:
            # ---- input DMAs: sync ring x/ctx, scalar ring weights.
            # high_priority pins them ahead of the (bulky, low-urgency)
            # transposed-x descriptor streams on the shared DMA queues.
            ctx_sb = bpool.tile([128, 2, CTX], BF16, name="ctx_sb")
            wq_sb = wpool.tile([128, DT, D], BF16, name="wq_sb")
            wk_sb = bpool.tile([128, KCH, D], BF16, name="wk_sb")
            wv_sb = bpool.tile([128, KCH, D], BF16, name="wv_sb")
            wo_sb = wpool.tile([128, DT, D], BF16, name="wo_sb")
            with tc.high_priority():
                nc.sync.dma_start(out=ctx_sb, in_=ctx_ext[:, :, :])
                nc.sync.dma_start(out=x01[:, 0:4, :], in_=x_pm[:, 0:4, :])
                nc.sync.dma_start(out=x01[:, 4:8, :], in_=x_pm[:, 4:8, :])
                nc.scalar.dma_start(out=wq_sb, in_=wq_ext[:, :, :])
                nc.scalar.dma_start(out=wk_sb, in_=wk_ext[:, :, :])
                nc.scalar.dma_start(out=wv_sb, in_=wv_ext[:, :, :])
                nc.scalar.dma_start(out=wo_sb, in_=wo_ext[:, :, :])
            # all transposed-x loads on the scalar ring AFTER the weights:
            # their huge descriptor streams must never sit in front of the
            # latency-critical small loads (the sync ring stays clean for
            # ctx/x01/out stores)
            for c23 in (2, 3):
                for k in range(DT):
                    nc.scalar.dma_start(
                        out=xT[:, k, ts(c23, QCH)],
                        in_=x_rm[ts(c23, QCH), ts(k, 128)], transpose=True,
                    )

            # ---- x^T for chunks 0-1 (PE transposes) + Q chunk 0 first:
            # their data arrives earliest, so they fill the PE queue while
            # the K/V weights are still streaming in
            def do_xT(c):
                for rr in range(4):
                    rb = 4 * c + rr
                    for kg in range(2):
                        tp = mpsum.tile([128, 4, 128], BF16, name="tp", tag="ps")
                        for kk in range(4):
                            nc.tensor.transpose(
                                tp[:, kk, :], x01[:, rb, ts(4 * kg + kk, 128)],
                                identb,
                            )
                        nc.vector.tensor_copy(
                            xT[:, 4 * kg : 4 * kg + 4, ts(rb, 128)], tp
                        )

            def do_qproj(c):
                cc = c % 2
                for m in range(DT):
                    ps = mpsum.tile([128, QCH], F32, name="ps_q", tag="ps")
                    for k in range(DT):
                        nc.tensor.matmul(
                            ps[:, :], wq_sb[:, k, ts(m, 128)],
                            xT[:, k, ts(c, QCH)],
                            start=(k == 0), stop=(k == DT - 1),
                        )
                    nc.scalar.activation(
                        qT[:, m, cc, :], ps,
                        mybir.ActivationFunctionType.Copy,
                    )

            do_xT(0)
            do_xT(1)
            do_qproj(0)

            # ---- K/V setup ------------------------------------------------
            ctxT = bpool.tile([128, KCH, M], BF16, name="ctxT")
            for a in range(2):
                tp = mpsum.tile([128, KCH, 128], BF16, name="tp_b", tag="ps")
                for k in range(KCH):
                    nc.tensor.transpose(
                        tp[:, k, :], ctx_sb[:, a, ts(k, 128)], identb
                    )
                nc.vector.tensor_copy(ctxT[:, :, ts(a, 128)], tp)
            for m in range(DT):
                ps = mpsum.tile([128, M], F32, name="ps_k", tag="ps")
                for k in range(KCH):
                    nc.tensor.matmul(
                        ps[:, :], wk_sb[:, k, ts(m, 128)], ctxT[:, k, :],
                        start=(k == 0), stop=(k == KCH - 1),
                    )
                nc.vector.tensor_copy(kT[:, m, :], ps)
            for j in range(2):
                for n in range(2):
                    psv = mpsum.tile([128, 8, HD], F32, name="ps_v", tag="ps")
                    for k in range(KCH):
                        nc.tensor.matmul(
                            psv[:, :, :], ctxT[:, k, ts(j, 128)],
                            wv_sb[:, k, ts(n, 512)],
                            start=(k == 0), stop=(k == KCH - 1),
                        )
                    nc.vector.tensor_copy(
                        vvx[:, j, 8 * n : 8 * n + 8, HD:128], psv
                    )

            # ---- main pipeline over q chunks ------------------------------
            for c in range(NQC):
                cc = c % 2
                # attention chunk c, per head pair
                for i in range(DT):
                    av_e = mpsum.tile([128, QCH], F32, name="av_e", tag="av_e", bufs=2)
                    av_o = mpsum.tile([128, QCH], F32, name="av_o", tag="av_o", bufs=2)
                    for j in range(2):  # kv chunk
                        sc_e = mpsum.tile([128, QCH], F32, name="sc_e", tag="ps")
                        sc_o = mpsum.tile([128, QCH], F32, name="sc_o", tag="ps")
                        nc.tensor.matmul(
                            sc_e[:, :], kT[0:64, i, ts(j, 128)],
                            qT[0:64, i, cc, :],
                            start=True, stop=True, tile_position=(0, 0),
                        )
                        nc.tensor.matmul(
                            sc_o[:, :], kT[64:128, i, ts(j, 128)],
                            qT[64:128, i, cc, :],
                            start=True, stop=True, tile_position=(64, 0),
                        )
                        at_e = attnp.tile([128, QCH], BF16, name="at_e", tag="at_e")
                        at_o = attnp.tile([128, QCH], BF16, name="at_o", tag="at_o")
                        nc.scalar.activation(
                            at_e, sc_e, mybir.ActivationFunctionType.Exp,
                            bias=mask_sb[:, j : j + 1], scale=SCALE,
                        )
                        nc.scalar.activation(
                            at_o, sc_o, mybir.ActivationFunctionType.Exp,
                            bias=mask_sb[:, j : j + 1], scale=SCALE,
                        )
                        nc.tensor.matmul(
                            av_e[:, :], vvx[:, j, 2 * i, :], at_e,
                            start=(j == 0), stop=(j == 1),
                        )
                        nc.tensor.matmul(
                            av_o[:, :], vvx[:, j, 2 * i + 1, :], at_o,
                            start=(j == 0), stop=(j == 1),
                        )
                    rec_e = recp.tile([64, QCH], F32, name="rec_e", tag="rec_e")
                    rec_o = recp.tile([64, QCH], F32, name="rec_o", tag="rec_o")
                    nc.vector.reciprocal_approx_fast(rec_e, av_e[0:64, :])
                    nc.vector.reciprocal_approx_fast(rec_o, av_o[0:64, :])
                    nc.vector.tensor_mul(
                        oT[0:64, i, cc, :], av_e[64:128, :], rec_e
                    )
                    nc.vector.tensor_mul(
                        oT[64:128, i, cc, :], av_o[64:128, :], rec_o
                    )
                # next chunk's Q projection between attention and out-proj
                if c + 1 < NQC:
                    do_qproj(c + 1)
                # out rows chunk c = O^T.T @ Wo, merged bf16 store
                ob = outp.tile([128, 4, D], BF16, name="ob", tag="ob")
                for mr in range(4):
                    for n in range(2):
                        ps = mpsum.tile([128, 512], F32, name="ps_o", tag="ps")
                        for k in range(DT):
                            nc.tensor.matmul(
                                ps[:, :], oT[:, k, cc, ts(mr, 128)],
                                wo_sb[:, k, ts(n, 512)],
                                start=(k == 0), stop=(k == DT - 1),
                            )
                        nc.vector.tensor_copy(ob[:, mr, ts(n, 512)], ps)
                nc.sync.dma_start(
                    out=out_ext[:, 4 * c : 4 * c + 4, :], in_=ob
                )

        # release singles in reverse allocation order
        free_oT()
        free_qT()
        free_xT()
        free_x01()
        free_vvx()
        free_kT()
        free_mask()
        free_identb()

    nc.finalize()
    return nc


_NC_CACHE = None


def _get_nc():
    global _NC_CACHE
    if _NC_CACHE is None:
        _NC_CACHE = build_nc()
    return _NC_CACHE


def _shuffle_pm(a, nblk):
    """[nblk*128, d] -> [128, nblk, d] partition-major."""
    d = a.shape[1]
    return np.ascontiguousarray(a.reshape(nblk, 128, d).transpose(1, 0, 2))


def make_in_maps(x, context, context_mask, Wq, Wk, Wv, Wo):
    import ml_dtypes

    bf = ml_dtypes.bfloat16
    x = np.asarray(x).astype(bf)
    context = np.asarray(context).astype(bf)
    Wq = np.asarray(Wq).astype(bf)
    Wk = np.asarray(Wk).astype(bf)
    Wv = np.asarray(Wv).astype(bf)
    Wo = np.asarray(Wo).astype(bf)
    mask = np.asarray(context_mask)

    # additive exp-bias per kv position: 0 where visible, -1e9 where masked
    bias = (mask.astype(np.float32) - 1.0) * 1e9          # [B, M]
    x_flat = x.reshape(B * N, D)

    wq_s = _shuffle_pm(Wq, DT)
    wk_s = _shuffle_pm(Wk, KCH)
    wv_s = _shuffle_pm(Wv, KCH)
    wo_s = _shuffle_pm(Wo, DT)

    in_maps = []
    for c in range(N_CORES):
        b = (c * NQ) // N
        shard = x_flat[c * NQ : (c + 1) * NQ]
        in_maps.append({
            "xpm": _shuffle_pm(shard[: 8 * 128], 8),
            "xrm": np.ascontiguousarray(shard),
            "ctx": _shuffle_pm(context[b], 2),
            "maskb": np.ascontiguousarray(bias[b].reshape(2, 128).T),
            "wq": wq_s, "wk": wk_s, "wv": wv_s, "wo": wo_s,
        })
    return in_maps


def kernel(x, context, context_mask, Wq, Wk, Wv, Wo):
    nc = _get_nc()
    in_maps = make_in_maps(x, context, context_mask, Wq, Wk, Wv, Wo)
    res = run_bass_kernel_spmd(nc, in_maps, core_ids=list(range(N_CORES)))
    # out arrives partition-major bf16: [128, NRB, D] per core
    outs = []
    for c in range(N_CORES):
        o = np.asarray(res.results[c]["out"], dtype=np.float32)
        outs.append(o.transpose(1, 0, 2).reshape(NQ, D))
    return np.concatenate(outs, axis=0).reshape(B, N, D)


# revision 22
# speedup vs baseline: 1.1369x; 1.0869x over previous
"""Cross-attention kernel for 8 TRN2 NeuronCores.

Reference computation (per problem spec):
    q = (x @ Wq)  [B=4, N=4096, D=1024] -> heads [B, 16, N, 64]
    k = (context @ Wk), v = (context @ Wv)   context [B, M=256, 768]
    out = softmax(q k^T / 8 + mask) v   -> [B, N, D] @ Wo

Sharding: the 16384 query rows (B*N) are split evenly across the 8 cores
(2048 rows each, each shard living inside one batch). K/V are computed
redundantly per core from that core's batch context (only ~0.8 GFLOP) so no
collectives are needed; each core produces its own 2048 output rows and the
host concatenates them.

On-core dataflow (bf16 matmuls, fp32 PSUM), pipelined over 512-row chunks:
  - host tensors arrive pre-shuffled partition-major ([128, ...]) so every
    big DMA is 128 large contiguous descriptors (descriptor throughput,
    not bytes, limits the rings here)
  - x^T: PE transposes for chunks 0-1 (fills the HAM warmup window),
    DMA-transposes on the two HWDGE rings for chunks 2-3 (row-major copy
    of x is passed separately for those)
  - Q^T = Wq^T x^T; scores^T = K Q^T (kv on partitions, two heads
    row-packed in the PE); exp on ScalarE with the mask as bias
  - AV+rowsum fused: lhsT = [ones | V_h] so one matmul per (head,
    kv-block) emits the softmax denominator on partitions 0:63
    (reciprocal_approx_fast needs base partition 0) and AV on 64:127;
    the normalize multiply mixes base partitions (PSUM+SB: allowed)
  - out rows = O^T.T Wo, evicted bf16 and stored once per chunk
"""

import sys

for _p in ("/opt/trn_rl_repo",):
    if _p not in sys.path:
        sys.path.insert(0, _p)

import numpy as np

import concourse.bass as bass
import concourse.mybir as mybir
import concourse.tile as tile
from concourse.masks import make_identity
from concourse import bacc
from concourse.bass_utils import run_bass_kernel_spmd

ts = bass.ts

N_CORES = 8
B, N, D = 4, 4096, 1024
CTX = 768
M = 256          # kv length
H, HD = 16, 64   # heads, head dim
NQ = (B * N) // N_CORES   # 2048 query rows per core
QCH = 512                 # q chunk (free dim of most matmuls)
NQC = NQ // QCH           # 4 q chunks
NRB = NQ // 128           # 16 row-blocks
DT = D // 128             # 8 d-blocks (= head pairs)
KCH = CTX // 128          # 6 contraction chunks for context projections
F32 = mybir.dt.float32
BF16 = mybir.dt.bfloat16

SCALE = HD ** -0.5


def build_nc():
    nc = bacc.Bacc()

    x_pm = nc.declare_dram_parameter("xpm", [128, NRB, D], BF16, isOutput=False)
    ctx_ext = nc.declare_dram_parameter("ctx", [128, 2, CTX], BF16, isOutput=False)
    maskb_ext = nc.declare_dram_parameter("maskb", [128, 2], F32, isOutput=False)
    wq_ext = nc.declare_dram_parameter("wq", [128, DT, D], BF16, isOutput=False)
    wk_ext = nc.declare_dram_parameter("wk", [128, KCH, D], BF16, isOutput=False)
    wv_ext = nc.declare_dram_parameter("wv", [128, KCH, D], BF16, isOutput=False)
    wo_ext = nc.declare_dram_parameter("wo", [128, DT, D], BF16, isOutput=False)
    out_ext = nc.declare_dram_parameter("out", [128, NRB, D], BF16, isOutput=True)

    with tile.TileContext(nc) as tc:
        # ---- persistent tensors -------------------------------------------
        identb, free_identb = tc.tile([128, 128], BF16, name="identb")
        make_identity(nc, identb)
        mask_sb, free_mask = tc.tile([128, 2], F32, name="mask_sb")
        nc.scalar.dma_start(out=mask_sb, in_=maskb_ext[:, :])

        kT, free_kT = tc.tile([128, DT, M], BF16, name="kT")
        # per (kv-block j, head h): [ones | V_h]; the ones columns make each
        # AV matmul also emit the softmax denominator on partitions 0:63
        vvx, free_vvx = tc.tile([128, 2, H, 128], BF16, name="vvx")
        x01, free_x01 = tc.tile([128, 8, D], BF16, name="x01")
        x23, free_x23 = tc.tile([128, 8, D], BF16, name="x23")
        xT, free_xT = tc.tile([128, DT, NQ], BF16, name="xT")
        qT, free_qT = tc.tile([128, DT, 2, QCH], BF16, name="qT")
        oT, free_oT = tc.tile([128, DT, 2, QCH], BF16, name="oT")

        nc.vector.memset(vvx, 1.0)

        with tc.tile_pool(name="weights", bufs=1) as wpool, \
             tc.tile_pool(name="bpool", bufs=1) as bpool, \
             tc.tile_pool(name="attnp", bufs=3) as attnp, \
             tc.tile_pool(name="recp", bufs=3) as recp, \
             tc.tile_pool(name="outp", bufs=2) as outp, \
             tc.tile_pool(name="mpsum", bufs=4, space="PSUM") as mpsum:
            # ---- input DMAs: sync ring x/ctx, scalar ring weights.
            # high_priority pins them ahead of the (bulky, low-urgency)
            # transposed-x descriptor streams on the shared DMA queues.
            ctx_sb = bpool.tile([128, 2, CTX], BF16, name="ctx_sb")
            wq_sb = wpool.tile([128, DT, D], BF16, name="wq_sb")
            wk_sb = bpool.tile([128, KCH, D], BF16, name="wk_sb")
            wv_sb = bpool.tile([128, KCH, D], BF16, name="wv_sb")
            wo_sb = wpool.tile([128, DT, D], BF16, name="wo_sb")
            with tc.high_priority():
                nc.sync.dma_start(out=x01[:, 0:4, :], in_=x_pm[:, 0:4, :])
                nc.sync.dma_start(out=x01[:, 4:8, :], in_=x_pm[:, 4:8, :])
                nc.sync.dma_start(out=ctx_sb, in_=ctx_ext[:, :, :])
                nc.scalar.dma_start(out=wq_sb, in_=wq_ext[:, :, :])
                nc.scalar.dma_start(out=wk_sb, in_=wk_ext[:, :, :])
                nc.scalar.dma_start(out=wv_sb, in_=wv_ext[:, :, :])
                nc.scalar.dma_start(out=wo_sb, in_=wo_ext[:, :, :])
            nc.sync.dma_start(out=x23[:, 0:4, :], in_=x_pm[:, 8:12, :])
            nc.sync.dma_start(out=x23[:, 4:8, :], in_=x_pm[:, 12:16, :])

            # ---- x^T for chunks 0-1 (PE transposes) + Q chunk 0 first:
            # their data arrives earliest, so they fill the PE queue while
            # the K/V weights are still streaming in
            def do_xT(c):
                xc = x01 if c < 2 else x23
                for rr in range(4):
                    rb = 4 * c + rr
                    for kg in range(2):
                        tp = mpsum.tile([128, 4, 128], BF16, name="tp", tag="ps")
                        for kk in range(4):
                            nc.tensor.transpose(
                                tp[:, kk, :],
                                xc[:, 4 * (c % 2) + rr, ts(4 * kg + kk, 128)],
                                identb,
                            )
                        nc.vector.tensor_copy(
                            xT[:, 4 * kg : 4 * kg + 4, ts(rb, 128)], tp
                        )

            def do_qproj(c):
                cc = c % 2
                for m in range(DT):
                    ps = mpsum.tile([128, QCH], F32, name="ps_q", tag="ps")
                    for k in range(DT):
                        nc.tensor.matmul(
                            ps[:, :], wq_sb[:, k, ts(m, 128)],
                            xT[:, k, ts(c, QCH)],
                            start=(k == 0), stop=(k == DT - 1),
                        )
                    nc.scalar.activation(
                        qT[:, m, cc, :], ps,
                        mybir.ActivationFunctionType.Copy,
                    )

            do_xT(0)
            do_xT(1)
            do_qproj(0)

            # ---- K/V setup ------------------------------------------------
            ctxT = bpool.tile([128, KCH, M], BF16, name="ctxT")
            for a in range(2):
                tp = mpsum.tile([128, KCH, 128], BF16, name="tp_b", tag="ps")
                for k in range(KCH):
                    nc.tensor.transpose(
                        tp[:, k, :], ctx_sb[:, a, ts(k, 128)], identb
                    )
                nc.vector.tensor_copy(ctxT[:, :, ts(a, 128)], tp)
            for m in range(DT):
                ps = mpsum.tile([128, M], F32, name="ps_k", tag="ps")
                for k in range(KCH):
                    nc.tensor.matmul(
                        ps[:, :], wk_sb[:, k, ts(m, 128)], ctxT[:, k, :],
                        start=(k == 0), stop=(k == KCH - 1),
                    )
                nc.vector.tensor_copy(kT[:, m, :], ps)
            for j in range(2):
                for n in range(2):
                    psv = mpsum.tile([128, 8, HD], F32, name="ps_v", tag="ps")
                    for k in range(KCH):
                        nc.tensor.matmul(
                            psv[:, :, :], ctxT[:, k, ts(j, 128)],
                            wv_sb[:, k, ts(n, 512)],
                            start=(k == 0), stop=(k == KCH - 1),
                        )
                    nc.vector.tensor_copy(
                        vvx[:, j, 8 * n : 8 * n + 8, HD:128], psv
                    )

            # ---- main pipeline over q chunks ------------------------------
            for c in range(NQC):
                cc = c % 2
                # attention chunk c, per head pair
                for i in range(DT):
                    av_e = mpsum.tile([128, QCH], F32, name="av_e", tag="av_e", bufs=2)
                    av_o = mpsum.tile([128, QCH], F32, name="av_o", tag="av_o", bufs=2)
                    for j in range(2):  # kv chunk
                        sc_e = mpsum.tile([128, QCH], F32, name="sc_e", tag="ps")
                        sc_o = mpsum.tile([128, QCH], F32, name="sc_o", tag="ps")
                        nc.tensor.matmul(
                            sc_e[:, :], kT[0:64, i, ts(j, 128)],
                            qT[0:64, i, cc, :],
                            start=True, stop=True, tile_position=(0, 0),
                        )
                        nc.tensor.matmul(
                            sc_o[:, :], kT[64:128, i, ts(j, 128)],
                            qT[64:128, i, cc, :],
                            start=True, stop=True, tile_position=(64, 0),
                        )
                        at_e = attnp.tile([128, QCH], BF16, name="at_e", tag="at_e")
                        at_o = attnp.tile([128, QCH], BF16, name="at_o", tag="at_o")
                        nc.scalar.activation(
                            at_e, sc_e, mybir.ActivationFunctionType.Exp,
                            bias=mask_sb[:, j : j + 1], scale=SCALE,
                        )
                        nc.scalar.activation(
                            at_o, sc_o, mybir.ActivationFunctionType.Exp,
                            bias=mask_sb[:, j : j + 1], scale=SCALE,
                        )
                        nc.tensor.matmul(
                            av_e[:, :], vvx[:, j, 2 * i, :], at_e,
                            start=(j == 0), stop=(j == 1),
                        )
                        nc.tensor.matmul(
                            av_o[:, :], vvx[:, j, 2 * i + 1, :], at_o,
                            start=(j == 0), stop=(j == 1),
                        )
                    rec_e = recp.tile([64, QCH], F32, name="rec_e", tag="rec_e")
                    rec_o = recp.tile([64, QCH], F32, name="rec_o", tag="rec_o")
                    nc.vector.reciprocal_approx_fast(rec_e, av_e[0:64, :])
                    nc.vector.reciprocal_approx_fast(rec_o, av_o[0:64, :])
                    nc.vector.tensor_mul(
                        oT[0:64, i, cc, :], av_e[64:128, :], rec_e
                    )
                    nc.vector.tensor_mul(
                        oT[64:128, i, cc, :], av_o[64:128, :], rec_o
                    )
                # next chunk's Q projection between attention and out-proj;
                # chunk c+2's x^T transposes ride along behind it
                if c + 1 < NQC:
                    do_qproj(c + 1)
                if c + 2 < NQC:
                    do_xT(c + 2)
                # out rows chunk c = O^T.T @ Wo, merged bf16 store
                ob = outp.tile([128, 4, D], BF16, name="ob", tag="ob")
                for mr in range(4):
                    for n in range(2):
                        ps = mpsum.tile([128, 512], F32, name="ps_o", tag="ps")
                        for k in range(DT):
                            nc.tensor.matmul(
                                ps[:, :], oT[:, k, cc, ts(mr, 128)],
                                wo_sb[:, k, ts(n, 512)],
                                start=(k == 0), stop=(k == DT - 1),
                            )
                        nc.vector.tensor_copy(ob[:, mr, ts(n, 512)], ps)
                nc.sync.dma_start(
                    out=out_ext[:, 4 * c : 4 * c + 4, :], in_=ob
                )

        # release singles in reverse allocation order
        free_oT()
        free_qT()
        free_xT()
        free_x23()
        free_x01()
        free_vvx()
        free_kT()
        free_mask()
        free_identb()

    nc.finalize()
    return nc


_NC_CACHE = None


def _get_nc():
    global _NC_CACHE
    if _NC_CACHE is None:
        _NC_CACHE = build_nc()
    return _NC_CACHE


def _shuffle_pm(a, nblk):
    """[nblk*128, d] -> [128, nblk, d] partition-major."""
    d = a.shape[1]
    return np.ascontiguousarray(a.reshape(nblk, 128, d).transpose(1, 0, 2))


def make_in_maps(x, context, context_mask, Wq, Wk, Wv, Wo):
    import ml_dtypes

    bf = ml_dtypes.bfloat16
    x = np.asarray(x).astype(bf)
    context = np.asarray(context).astype(bf)
    Wq = np.asarray(Wq).astype(bf)
    Wk = np.asarray(Wk).astype(bf)
    Wv = np.asarray(Wv).astype(bf)
    Wo = np.asarray(Wo).astype(bf)
    mask = np.asarray(context_mask)

    # additive exp-bias per kv position: 0 where visible, -1e9 where masked
    bias = (mask.astype(np.float32) - 1.0) * 1e9          # [B, M]
    x_flat = x.reshape(B * N, D)

    wq_s = _shuffle_pm(Wq, DT)
    wk_s = _shuffle_pm(Wk, KCH)
    wv_s = _shuffle_pm(Wv, KCH)
    wo_s = _shuffle_pm(Wo, DT)

    in_maps = []
    for c in range(N_CORES):
        b = (c * NQ) // N
        shard = x_flat[c * NQ : (c + 1) * NQ]
        in_maps.append({
            "xpm": _shuffle_pm(shard, NRB),
            "ctx": _shuffle_pm(context[b], 2),
            "maskb": np.ascontiguousarray(bias[b].reshape(2, 128).T),
            "wq": wq_s, "wk": wk_s, "wv": wv_s, "wo": wo_s,
        })
    return in_maps


def kernel(x, context, context_mask, Wq, Wk, Wv, Wo):
    nc = _get_nc()
    in_maps = make_in_maps(x, context, context_mask, Wq, Wk, Wv, Wo)
    res = run_bass_kernel_spmd(nc, in_maps, core_ids=list(range(N_CORES)))
    # out arrives partition-major bf16: [128, NRB, D] per core
    outs = []
    for c in range(N_CORES):
        o = np.asarray(res.results[c]["out"], dtype=np.float32)
        outs.append(o.transpose(1, 0, 2).reshape(NQ, D))
    return np.concatenate(outs, axis=0).reshape(B, N, D)
